# revision 1
# baseline (speedup 1.0000x reference)
"""Trainium2 Bass kernel for nn_Attention_Module (dense_transformer).

Data-parallel over batch: B=64 split across 8 NeuronCores (8 per core).
Per core, all activations are kept channel-major [C, tokens] with the
8 local batches' 320 tokens reordered into a z-block (8*64=512 template
tokens) followed by an x-block (8*256=2048 search tokens), processed as
5 token-tiles of 512.  Matmuls run as float32r (full-rate fp32 on the
PE with ~1e-4 rounding), everything else fp32.

Self-contained: only imports infra from /opt/trn_rl_repo.
"""
import sys

sys.path.insert(0, "/opt/trn_rl_repo")

from contextlib import ExitStack

import numpy as np

import concourse.bacc as bacc
import concourse.tile as tile
from concourse import mybir
F32 = mybir.dt.float32
F32R = mybir.dt.float32r
AF = mybir.ActivationFunctionType
OP = mybir.AluOpType
AX = mybir.AxisListType

B_LOC = 8          # batches per core
DIM = 512
HID = 256
HEADS = 8
NZ, NX = 64, 256   # template / search tokens per batch
NTOK = NZ + NX     # 320
T = B_LOC * NTOK   # 2560 reordered tokens per core
NT = 5             # token tiles of 512
EPS_LN = 1e-5
TINY = 1e-24       # guards rsqrt of exact-zero row norms


def _bbs(j):
    """Branch segments inside token-tile j: list of (batch, col_off, width).

    Tile 0 is the z-block (8 branches of 64), tiles 1..4 hold two x-branches
    of 256 tokens each (batches 2j-2 and 2j-1).
    """
    if j == 0:
        return [(b, 64 * b, 64) for b in range(B_LOC)]
    return [(2 * (j - 1), 0, 256), (2 * j - 1, 256, 256)]


def build_nc():
    nc = bacc.Bacc("TRN2", target_bir_lowering=False, debug=False,
                   num_devices=8)

    # ---- DRAM I/O (per-core shapes) ----
    x1_e = nc.declare_dram_parameter("x1", [B_LOC, DIM, NTOK], F32R, isOutput=False)
    x2_e = nc.declare_dram_parameter("x2", [B_LOC, DIM, NTOK], F32R, isOutput=False)
    out_e = nc.declare_dram_parameter("out", [B_LOC, DIM, NTOK], F32, isOutput=True)
    wlin_e = nc.declare_dram_parameter("W_lin", [DIM, 2 * DIM], F32R, isOutput=False)
    wdown_e = nc.declare_dram_parameter("W_down", [DIM, HID], F32R, isOutput=False)
    wup_e = nc.declare_dram_parameter("W_up", [HID, DIM], F32R, isOutput=False)
    wq_e = nc.declare_dram_parameter("WqT", [HID, HID], F32R, isOutput=False)
    wk_e = nc.declare_dram_parameter("WkT", [HID, HID], F32R, isOutput=False)
    wv_e = nc.declare_dram_parameter("WvT", [HID, HID], F32R, isOutput=False)
    wo_e = nc.declare_dram_parameter("WoT", [HID, HID], F32R, isOutput=False)
    wend_e = nc.declare_dram_parameter("W_end", [DIM, DIM], F32R, isOutput=False)
    blin_e = nc.declare_dram_parameter("b_lin", [2 * DIM], F32, isOutput=False)
    bdown_e = nc.declare_dram_parameter("b_down", [HID], F32, isOutput=False)
    bup_e = nc.declare_dram_parameter("b_up", [DIM], F32, isOutput=False)
    bend_e = nc.declare_dram_parameter("b_end", [DIM], F32, isOutput=False)
    gamma_e = nc.declare_dram_parameter("gamma", [DIM], F32, isOutput=False)
    beta_e = nc.declare_dram_parameter("beta", [DIM], F32, isOutput=False)
    temp_e = nc.declare_dram_parameter("temp_col", [128, 2], F32, isOutput=False)
    ones_e = nc.declare_dram_parameter("ones_in", [128, 128], F32R, isOutput=False)
    ident_e = nc.declare_dram_parameter("ident_in", [128, 128], F32R, isOutput=False)
    zeros_e = nc.declare_dram_parameter("zeros_in", [128, 4, 128], F32R, isOutput=False)

    x1r = x1_e.rearrange("b c t -> c b t")
    x2r = x2_e.rearrange("b c t -> c b t")
    outr = out_e.rearrange("b c t -> c b t")

    with tile.TileContext(nc) as tc, ExitStack() as ctx:
        wts = ctx.enter_context(tc.tile_pool(name="wts", bufs=1))
        xload = ctx.enter_context(tc.tile_pool(name="xload", bufs=3))
        u1p = ctx.enter_context(tc.tile_pool(name="u1p", bufs=1))
        rp = ctx.enter_context(tc.tile_pool(name="rp", bufs=2))
        u2p = ctx.enter_context(tc.tile_pool(name="u2p", bufs=1))
        ap_ = ctx.enter_context(tc.tile_pool(name="ap", bufs=2))
        bqp = ctx.enter_context(tc.tile_pool(name="bqp", bufs=1))
        qkvp = ctx.enter_context(tc.tile_pool(name="qkvp", bufs=2))
        sqp = ctx.enter_context(tc.tile_pool(name="sqp", bufs=2))
        nrmp = ctx.enter_context(tc.tile_pool(name="nrmp", bufs=2))
        qtp = ctx.enter_context(tc.tile_pool(name="qtp", bufs=1))
        ep = ctx.enter_context(tc.tile_pool(name="ep", bufs=1))
        avp = ctx.enter_context(tc.tile_pool(name="avp", bufs=1))
        o1p = ctx.enter_context(tc.tile_pool(name="o1p", bufs=1))
        scr = ctx.enter_context(tc.tile_pool(name="scr", bufs=1))
        yp = ctx.enter_context(tc.tile_pool(name="yp", bufs=1))
        prep = ctx.enter_context(tc.tile_pool(name="prep", bufs=1))
        statp = ctx.enter_context(tc.tile_pool(name="statp", bufs=1))
        outp = ctx.enter_context(tc.tile_pool(name="outp", bufs=1))
        ps = ctx.enter_context(tc.tile_pool(name="ps", bufs=4, space="PSUM"))
        psg = ctx.enter_context(tc.tile_pool(name="psg", bufs=3, space="PSUM"))
        psav = ctx.enter_context(tc.tile_pool(name="psav", bufs=1, space="PSUM"))

        # ---- weights / constants to SBUF ----
        wlin_sb = wts.tile([128, 4, 2 * DIM], F32R)
        wlin_r = wlin_e.rearrange("(kt p) m -> p kt m", p=128)
        for kt_ in range(4):
            nc.sync.dma_start(wlin_sb[:, kt_, 512:], wlin_r[:, kt_, 512:])
        for kt_ in range(4):
            nc.sync.dma_start(wlin_sb[:, kt_, 0:512], wlin_r[:, kt_, 0:512])
        wdown_sb = wts.tile([128, 4, HID], F32R)
        wup_sb = wts.tile([128, 2, DIM], F32R)
        wq_sb = wts.tile([128, 2, HID], F32R)
        wk_sb = wts.tile([128, 2, HID], F32R)
        wv_sb = wts.tile([128, 2, HID], F32R)
        wo_sb = wts.tile([128, 2, HID], F32R)
        wend_sb = wts.tile([128, 4, DIM], F32R)

        blin_sb = wts.tile([128, 8], F32)
        nc.sync.dma_start(blin_sb[:], blin_e.rearrange("(m p) -> p m", p=128))
        bdown_sb = wts.tile([128, 2], F32)
        bup_sb = wts.tile([128, 4], F32)
        bend_sb = wts.tile([128, 4], F32)
        gamma_sb = wts.tile([128, 4], F32)
        beta_sb = wts.tile([128, 4], F32)
        tempc_sb = wts.tile([128, 2], F32)
        nc.sync.dma_start(tempc_sb[:], temp_e[:, :])

        ones_sb = wts.tile([128, 128], F32R)
        ident_sb = wts.tile([128, 128], F32R)
        bd = wts.tile([128, 4, 128], F32R)
        tiny_sb = wts.tile([128, 1], F32)
        nc.vector.memset(tiny_sb[:], TINY)
        epsln_sb = wts.tile([128, 1], F32)
        nc.vector.memset(epsln_sb[:], EPS_LN)

        def emit_loads(j):
            bbs = _bbs(j)

            # ---- S1: load X1/X2 (token-reordered) ----
            x1t = xload.tile([128, 4, 512], F32R, tag="xl")
            x2t = xload.tile([128, 4, 512], F32R, tag="xl")
            for tsr, src in ((x1t, x1r), (x2t, x2r)):
                for kt in range(4):
                    cs = slice(128 * kt, 128 * (kt + 1))
                    if j == 0:
                        nc.sync.dma_start(
                            tsr[:, kt, :].rearrange("p (b t) -> p b t", b=8),
                            src[cs, :, 0:64])
                    else:
                        bs = slice(2 * (j - 1), 2 * j)
                        nc.sync.dma_start(
                            tsr[:, kt, :].rearrange("p (b t) -> p b t", b=2),
                            src[cs, bs, 64:320])

            return (x1t, x2t)

        def emit_front(j, ld):
            bbs = _bbs(j)
            nb = len(bbs)
            x1t, x2t = ld
            # ---- S1: h1 = relu(W_lin^T X1 + b); u1 kept, r = y1 + u1 ----
            u1 = u1p.tile([128, 4, 512], F32R)
            r = rp.tile([128, 4, 512], F32)
            for m in [4, 5, 6, 7, 0, 1, 2, 3]:
                pt = ps.tile([128, 512], F32, tag="ps")
                for kt in range(4):
                    nc.tensor.matmul(pt[:], wlin_sb[:, kt, 128 * m:128 * (m + 1)],
                                     x1t[:, kt, :], start=(kt == 0), stop=(kt == 3))
                if m >= 4:
                    nc.scalar.activation(u1[:, m - 4, :], pt[:], AF.Relu,
                                         bias=blin_sb[:, m:m + 1])
                else:
                    ytmp = scr.tile([128, 512], F32, tag="ytmp")
                    nc.scalar.activation(ytmp[:], pt[:], AF.Relu,
                                         bias=blin_sb[:, m:m + 1])
                    nc.gpsimd.tensor_add(r[:, m, :], ytmp[:], u1[:, m, :])

            # ---- S1b: u2 = relu(W_lin[:,512:]^T X2 + b2) ----
            u2 = u2p.tile([128, 4, 512], F32R)
            for m in range(4):
                pt = ps.tile([128, 512], F32, tag="ps")
                for kt in range(4):
                    nc.tensor.matmul(
                        pt[:], wlin_sb[:, kt, 512 + 128 * m:512 + 128 * (m + 1)],
                        x2t[:, kt, :], start=(kt == 0), stop=(kt == 3))
                nc.scalar.activation(u2[:, m, :], pt[:], AF.Relu,
                                     bias=blin_sb[:, 4 + m:5 + m])

            # ---- S2: A = relu(W_down^T u1 + b_down); Bq likewise from u2 ----
            A = ap_.tile([128, 2, 512], F32R)
            Bq = bqp.tile([128, 2, 512], F32R)
            for (dst, src) in ((A, u1), (Bq, u2)):
                for m in range(2):
                    pt = ps.tile([128, 512], F32, tag="ps")
                    for kt in range(4):
                        nc.tensor.matmul(pt[:],
                                         wdown_sb[:, kt, 128 * m:128 * (m + 1)],
                                         src[:, kt, :],
                                         start=(kt == 0), stop=(kt == 3))
                    nc.scalar.activation(dst[:, m, :], pt[:], AF.Relu,
                                         bias=bdown_sb[:, m:m + 1])

            # ---- S3: q = Wq@Bq, k = Wk@A, v = Wv@A (channel-major) ----
            q = qkvp.tile([128, 2, 512], F32R, tag="q")
            k = qkvp.tile([128, 2, 512], F32R, tag="k")
            v = qkvp.tile([128, 2, 512], F32R, tag="v")
            for (dst, w_sb, src) in ((q, wq_sb, Bq), (k, wk_sb, A), (v, wv_sb, A)):
                for m in range(2):
                    pt = ps.tile([128, 512], F32, tag="ps")
                    for kt in range(2):
                        nc.tensor.matmul(pt[:], w_sb[:, kt, 128 * m:128 * (m + 1)],
                                         src[:, kt, :],
                                         start=(kt == 0), stop=(kt == 1))
                    nc.vector.tensor_copy(dst[:, m, :], pt[:])

            return dict(x1t=x1t, r=r, A=A, q=q, k=k, v=v)

        def emit_back(j, st):
            bbs = _bbs(j)
            nb = len(bbs)
            x1t, r, A = st["x1t"], st["r"], st["A"]
            q, k, v = st["q"], st["k"], st["v"]
            # ---- S4: per-branch L2 row norms; normalize q,k in place ----
            # rnq additionally carries temperature.
            rn = {}
            for (name, t_) in (("q", q), ("k", k)):
                sq = sqp.tile([128, 2, 512], F32, tag="sq")
                nc.vector.tensor_mul(sq[:], t_[:], t_[:])
                ssq = nrmp.tile([128, 2, nb], F32, tag="ssq" + name)
                w = 512 // nb
                nc.vector.reduce_sum(
                    ssq[:], sq[:].rearrange("p g (n w) -> p g n w", w=w), axis=AX.X)
                rr = nrmp.tile([128, 2, nb], F32, tag="rn" + name)
                nc.scalar.activation(rr[:], ssq[:], AF.Sqrt, bias=tiny_sb[:, 0:1])
                nc.vector.reciprocal(rr[:], rr[:])
                rn[name] = rr
            for g in range(2):
                nc.vector.tensor_scalar_mul(rn["q"][:, g, :], in0=rn["q"][:, g, :],
                                            scalar1=tempc_sb[:, g:g + 1])

            rr = rn["k"]
            for g in range(2):
                for bi, (b, off, w) in enumerate(bbs):
                    nc.vector.tensor_scalar_mul(
                        k[:, g, off:off + w], in0=k[:, g, off:off + w],
                        scalar1=rr[:, g, bi:bi + 1])

            # ---- S5: PE-transpose qn,kn -> token-major qT,kT ----
            qT = qtp.tile([128, 4, 256], F32R, tag="qT")
            kT = qtp.tile([128, 4, 256], F32R, tag="kT")
            for (dst, src) in ((qT, q), (kT, k)):
                for tb in range(4):
                    pt = ps.tile([128, 256], F32R, tag="ps")
                    for g in range(2):
                        nc.tensor.matmul(
                            pt[:, 128 * g:128 * (g + 1)],
                            src[:, g, 128 * tb:128 * (tb + 1)], ident_sb[:],
                            is_transpose=True, start=(g == 0), stop=(g == 1))
                    nc.vector.tensor_copy(dst[:, tb, :], pt[:])

            # ---- S6-S8: per-branch attention pipeline (no all-branch
            # barrier): G -> exp(diag blocks) -> S/R -> blockwise-T -> AV.
            E = ep.tile([128, 2, 32 * nb], F32, tag="E")
            ET = ep.tile([128, 2, 32 * nb], F32, tag="ET")
            S = nrmp.tile([128, 2, nb], F32, tag="S")
            R = nrmp.tile([128, 2, nb], F32, tag="R")
            av = avp.tile([128, 2, 512], F32R)
            for bi, (b, off, w) in enumerate(bbs):
                if j == 0:
                    chunks = [(off // 128, off % 128, 64)]
                else:
                    chunks = [(off // 128, 0, 128), (off // 128 + 1, 0, 128)]
                gps = [psg.tile([128, 256], F32, tag="gps", name=f"gps{g_}")
                       for g_ in range(2)]
                for g in range(2):
                    for ci, (tb, tpo, cw) in enumerate(chunks):
                        nc.tensor.matmul(
                            gps[g][:, :],
                            qT[tpo:tpo + cw, tb, 128 * g:128 * (g + 1)],
                            kT[tpo:tpo + cw, tb, :],
                            start=(ci == 0), stop=(ci == len(chunks) - 1))
                for g in range(2):
                    for pos in range(4):
                        h = 4 * g + pos
                        nc.scalar.activation(
                            E[32 * pos:32 * (pos + 1), g, 32 * bi:32 * (bi + 1)],
                            gps[g][32 * pos:32 * (pos + 1), 32 * h:32 * (h + 1)],
                            AF.Exp,
                            scale=rn["q"][32 * pos:32 * (pos + 1), g, bi:bi + 1])
                nc.vector.reduce_sum(
                    S[:, :, bi:bi + 1],
                    E[:, :, 32 * bi:32 * (bi + 1)].rearrange(
                        "p g (n w) -> p g n w", w=32), axis=AX.X)
                nc.vector.reciprocal(R[:, :, bi:bi + 1], S[:, :, bi:bi + 1])
                pav = [psav.tile([128, 512], F32, tag="pav", name=f"pav{g_}")
                       for g_ in range(2)]
                for g in range(2):
                    bsl = 2 * (bi % 2) + g
                    nc.vector.transpose(ET[:, g, 32 * bi:32 * (bi + 1)],
                                        E[:, g, 32 * bi:32 * (bi + 1)])
                    for pos in range(4):
                        nc.vector.tensor_copy(
                            bd[32 * pos:32 * (pos + 1), bsl, 32 * pos:32 * (pos + 1)],
                            ET[32 * pos:32 * (pos + 1), g, 32 * bi:32 * (bi + 1)])
                    nc.tensor.matmul(
                        pav[g][:, off:off + w], bd[:, bsl, :],
                        v[:, g, off:off + w], start=True, stop=True)
                for g in range(2):
                    nc.vector.tensor_scalar_mul(
                        av[:, g, off:off + w], in0=pav[g][:, off:off + w],
                        scalar1=R[:, g, bi:bi + 1])

            # ---- S9: o1 = Wo@av + A (res1) ----
            o1 = o1p.tile([128, 2, 512], F32R)
            for m in range(2):
                pt = ps.tile([128, 512], F32, tag="ps")
                for kt in range(2):
                    nc.tensor.matmul(pt[:], wo_sb[:, kt, 128 * m:128 * (m + 1)],
                                     av[:, kt, :], start=(kt == 0), stop=(kt == 1))
                nc.vector.tensor_add(o1[:, m, :], pt[:], A[:, m, :])

            # ---- S10: y = W_up^T o1 + b_up + r ----
            y = yp.tile([128, 4, 512], F32R)
            for m in range(4):
                pt = ps.tile([128, 512], F32, tag="ps")
                for kt in range(2):
                    nc.tensor.matmul(pt[:], wup_sb[:, kt, 128 * m:128 * (m + 1)],
                                     o1[:, kt, :], start=(kt == 0), stop=(kt == 1))
                nc.vector.scalar_tensor_tensor(
                    y[:, m, :], in0=pt[:], scalar=bup_sb[:, m:m + 1],
                    in1=r[:, m, :], op0=OP.add, op1=OP.add)

            # ---- S11: pre = W_end^T y + b_end + t1 ; LN stats via ones-matmul
            pre = prep.tile([128, 4, 512], F32R)
            s1ps = ps.tile([128, 512], F32, tag="ps")
            s2ps = ps.tile([128, 512], F32, tag="ps")
            for m in range(4):
                pt = ps.tile([128, 512], F32, tag="ps")
                for kt in range(4):
                    nc.tensor.matmul(pt[:], wend_sb[:, kt, 128 * m:128 * (m + 1)],
                                     y[:, kt, :], start=(kt == 0), stop=(kt == 3))
                nc.vector.scalar_tensor_tensor(
                    pre[:, m, :], in0=pt[:], scalar=bend_sb[:, m:m + 1],
                    in1=x1t[:, m, :], op0=OP.add, op1=OP.add)
                p2 = scr.tile([128, 512], F32R, tag="p2")
                nc.vector.tensor_mul(p2[:], pre[:, m, :], pre[:, m, :])
                nc.tensor.matmul(s1ps[:], ones_sb[:], pre[:, m, :],
                                 start=(m == 0), stop=(m == 3))
                nc.tensor.matmul(s2ps[:], ones_sb[:], p2[:],
                                 start=(m == 0), stop=(m == 3))

            # ---- S12: mu/rstd (rows replicated) ----
            mu = statp.tile([128, 512], F32, tag="mu")
            nc.vector.tensor_scalar_mul(mu[:], in0=s1ps[:], scalar1=1.0 / DIM)
            var = statp.tile([128, 512], F32, tag="var")
            nc.vector.tensor_mul(var[:], mu[:], mu[:])
            nc.vector.scalar_tensor_tensor(var[:], in0=s2ps[:], scalar=1.0 / DIM,
                                           in1=var[:], op0=OP.mult, op1=OP.subtract)
            rstd = statp.tile([128, 512], F32, tag="rstd")
            nc.scalar.activation(rstd[:], var[:], AF.Sqrt, bias=epsln_sb[:, 0:1])
            nc.vector.reciprocal(rstd[:], rstd[:])

            # ---- S13: out = (pre*gamma)*rstd - (mur*gamma - beta) ----
            ot = outp.tile([128, 4, 512], F32)
            for m in range(4):
                t1 = scr.tile([128, 512], F32, tag="t1")
                nc.vector.tensor_sub(t1[:], pre[:, m, :], mu[:])
                mgb = scr.tile([128, 512], F32, tag="mgb")
                nc.vector.tensor_mul(mgb[:], t1[:], rstd[:])
                nc.vector.tensor_scalar(
                    ot[:, m, :], in0=mgb[:], scalar1=gamma_sb[:, m:m + 1],
                    scalar2=beta_sb[:, m:m + 1], op0=OP.mult, op1=OP.add)

            # ---- S14: store (un-reorder tokens) ----
            for m in range(4):
                cs = slice(128 * m, 128 * (m + 1))
                if j == 0:
                    nc.sync.dma_start(
                        outr[cs, :, 0:64],
                        ot[:, m, :].rearrange("p (b t) -> p b t", b=8))
                else:
                    bs = slice(2 * (j - 1), 2 * j)
                    nc.sync.dma_start(
                        outr[cs, bs, 64:320],
                        ot[:, m, :].rearrange("p (b t) -> p b t", b=2))

        prev = None
        order = [1, 2, 0, 3, 4]
        for j in order:
            ld = emit_loads(j)
            if j == order[0]:
                nc.sync.dma_start(wdown_sb[:], wdown_e.rearrange("(kt p) m -> p kt m", p=128))
                nc.sync.dma_start(wq_sb[:], wq_e.rearrange("(kt p) m -> p kt m", p=128))
                nc.sync.dma_start(wk_sb[:], wk_e.rearrange("(kt p) m -> p kt m", p=128))
                nc.sync.dma_start(wv_sb[:], wv_e.rearrange("(kt p) m -> p kt m", p=128))
                nc.sync.dma_start(bdown_sb[:], bdown_e.rearrange("(m p) -> p m", p=128))
                nc.sync.dma_start(wup_sb[:], wup_e.rearrange("(kt p) m -> p kt m", p=128))
                nc.sync.dma_start(wo_sb[:], wo_e.rearrange("(kt p) m -> p kt m", p=128))
                nc.sync.dma_start(wend_sb[:], wend_e.rearrange("(kt p) m -> p kt m", p=128))
                nc.sync.dma_start(bup_sb[:], bup_e.rearrange("(m p) -> p m", p=128))
                nc.sync.dma_start(bend_sb[:], bend_e.rearrange("(m p) -> p m", p=128))
                nc.sync.dma_start(gamma_sb[:], gamma_e.rearrange("(m p) -> p m", p=128))
                nc.sync.dma_start(beta_sb[:], beta_e.rearrange("(m p) -> p m", p=128))
                nc.sync.dma_start(ones_sb[:], ones_e[:, :])
                nc.sync.dma_start(ident_sb[:], ident_e[:, :])
                nc.sync.dma_start(bd[:], zeros_e[:, :, :])
            st = emit_front(j, ld)
            if prev is not None:
                emit_back(prev[0], prev[1])
            prev = (j, st)
        emit_back(prev[0], prev[1])

    nc.compile()
    return nc


# ---------------- host side ----------------
_CACHE = {}


def _get_runner():
    if "runner" in _CACHE:
        return _CACHE["runner"]
    import jax
    from jax.sharding import Mesh, PartitionSpec
    from jax.experimental.shard_map import shard_map
    from concourse.bass2jax import (
        _bass_exec_p, install_neuronx_cc_hook, partition_id_tensor)
    import concourse.mybir as mybir_

    nc = build_nc()
    install_neuronx_cc_hook()
    partition_name = nc.partition_id_tensor.name if nc.partition_id_tensor else None
    in_names, out_names, out_avals, zero_outs = [], [], [], []
    for alloc in nc.m.functions[0].allocations:
        if not isinstance(alloc, mybir_.MemoryLocationSet):
            continue
        name = alloc.memorylocations[0].name
        if alloc.kind == "ExternalInput":
            if name != partition_name:
                in_names.append(name)
        elif alloc.kind == "ExternalOutput":
            out_names.append(name)
            shape = tuple(alloc.tensor_shape)
            dtype = mybir_.dt.np(alloc.dtype)
            out_avals.append(jax.core.ShapedArray(shape, dtype))
            zero_outs.append(np.zeros(shape, dtype))
    n_params, n_outs = len(in_names), len(out_avals)
    all_in = list(in_names) + list(out_names)
    if partition_name is not None:
        all_in.append(partition_name)
    donate = tuple(range(n_params, n_params + n_outs))

    def _body(*args):
        operands = list(args)
        if partition_name is not None:
            operands.append(partition_id_tensor())
        return tuple(_bass_exec_p.bind(
            *operands, out_avals=tuple(out_avals), in_names=tuple(all_in),
            out_names=tuple(out_names), lowering_input_output_aliases=(),
            sim_require_finite=True, sim_require_nnan=True, nc=nc))

    devices = jax.devices()[:8]
    mesh = Mesh(np.asarray(devices), ("core",))
    fn = jax.jit(
        shard_map(_body, mesh=mesh,
                  in_specs=(PartitionSpec("core"),) * (n_params + n_outs),
                  out_specs=(PartitionSpec("core"),) * n_outs,
                  check_rep=False),
        donate_argnums=donate, keep_unused=True)
    _CACHE["runner"] = (fn, in_names, out_names, out_avals, zero_outs)
    return _CACHE["runner"]


def _prep_inputs(inputs):
    f = lambda a: np.ascontiguousarray(np.asarray(a), dtype=np.float32)
    x1 = f(inputs["x1"]).reshape(64, DIM, NTOK)
    x2 = f(inputs["x2"]).reshape(64, DIM, NTOK)
    temp = f(inputs["temperature"]).reshape(HEADS)
    # temp_col[p, g] = temperature[4*g + p//32]
    temp_col = np.empty((128, 2), np.float32)
    for g in range(2):
        for hh in range(4):
            temp_col[32 * hh:32 * (hh + 1), g] = temp[4 * g + hh]
    shared = {
        "W_lin": f(inputs["W_lin"]), "W_down": f(inputs["W_down"]),
        "W_up": f(inputs["W_up"]),
        "WqT": np.ascontiguousarray(f(inputs["Wq"]).T),
        "WkT": np.ascontiguousarray(f(inputs["Wk"]).T),
        "WvT": np.ascontiguousarray(f(inputs["Wv"]).T),
        "WoT": np.ascontiguousarray(f(inputs["Wo"]).T),
        "W_end": f(inputs["W_end"]), "b_lin": f(inputs["b_lin"]),
        "b_down": f(inputs["b_down"]), "b_up": f(inputs["b_up"]),
        "b_end": f(inputs["b_end"]), "gamma": f(inputs["gamma"]),
        "beta": f(inputs["beta"]), "temp_col": temp_col,
        "ones_in": np.ones((128, 128), np.float32),
        "ident_in": np.eye(128, dtype=np.float32),
        "zeros_in": np.zeros((128, 4, 128), np.float32),
    }
    in_maps = []
    for c in range(8):
        m = dict(shared)
        m["x1"] = np.ascontiguousarray(x1[8 * c:8 * (c + 1)])
        m["x2"] = np.ascontiguousarray(x2[8 * c:8 * (c + 1)])
        in_maps.append(m)
    return in_maps


def run_in_maps(in_maps):
    """Run the prebuilt executable on 8 cores; returns per-core out arrays."""
    import jax
    fn, in_names, out_names, out_avals, zero_outs = _get_runner()
    per_core = [[np.asarray(m[name]) for name in in_names] for m in in_maps]
    concat_in = [np.concatenate([per_core[c][i] for c in range(8)], axis=0)
                 for i in range(len(in_names))]
    concat_zeros = [np.zeros((8 * z.shape[0], *z.shape[1:]), z.dtype)
                    for z in zero_outs]
    out = fn(*concat_in, *concat_zeros)
    jax.block_until_ready(out)
    oi = out_names.index("out")
    arr = np.asarray(out[oi]).reshape(8, *out_avals[oi].shape)
    return arr


def kernel(**inputs):
    in_maps = _prep_inputs(inputs)
    arr = run_in_maps(in_maps)  # [8, 8, 512, 320]
    full = arr.reshape(64, DIM, NTOK).reshape(64, DIM, 16, 20)
    return full.astype(np.float32)


if __name__ == "__main__":
    rng = np.random.default_rng(0)
    ins = {
        "x1": rng.standard_normal((64, 512, 16, 20), dtype=np.float32),
        "x2": rng.standard_normal((64, 512, 16, 20), dtype=np.float32),
    }
    s = 0.02
    for nm, shape in [("W_lin", (512, 1024)), ("W_down", (512, 256)),
                      ("W_up", (256, 512)), ("Wq", (256, 256)),
                      ("Wk", (256, 256)), ("Wv", (256, 256)),
                      ("Wo", (256, 256)), ("W_end", (512, 512))]:
        ins[nm] = (rng.standard_normal(shape) * s).astype(np.float32)
    for nm, n in [("b_lin", 1024), ("b_down", 256), ("b_up", 512),
                  ("b_end", 512)]:
        ins[nm] = np.zeros(n, np.float32)
    ins["gamma"] = np.ones(512, np.float32)
    ins["beta"] = np.zeros(512, np.float32)
    ins["temperature"] = np.ones((8, 1, 1), np.float32)
    out = kernel(**ins)
    print("kernel ran, out shape", out.shape, "mean", float(np.abs(out).mean()))

